# revision 36
# baseline (speedup 1.0000x reference)
"""MultiHeadGeneralizedPooling Trainium2 kernel.

Data-parallel over batch: 32 batches -> 8 cores x 4 batches.
Per core, everything is computed in "feature-major" layout (feature dim on
SBUF partitions, sequence on the free axis):

  Hi^T (d, s)  = P_cat^T @ X^T          TensorE bf16, PSUM; +P_b on copy-out
  A1^T (dh, s) = relu(W1aug^T @ Hi^T)   K=97 (97th row = ones -> W1 bias)
  A2^T (d, s)  = W2^T @ A1^T            accumulated over 3 k-tiles
  E            = exp(A2^T + W2_b)       ScalarE; accum_out -> Z per partition
  u[d]         = sum_s E * Hi           fused DVE tensor_tensor_reduce accum
  v            = u / Z                  one tiny (96, 32) multiply at the end

Host side pre-transposes/casts X to X^T bf16 and packs the (tiny) weights
into lhsT layouts, so the device does no transposes of the big tensor.
"""

import numpy as np
from contextlib import ExitStack

B, S, T = 32, 2048, 768
NH, DH, DHID = 8, 96, 384
NCORES = 8
BPC = B // NCORES  # batches per core
KT = T // 128      # 6 contraction tiles
DT = (NH * DH) // 128  # 6 d-tiles of the packed head dim
SC = 4             # s-chunks per batch
SCW = S // SC      # 512
KC = DHID // 128   # 3
import os
N_WARM = int(os.environ.get("K_NWARM", "45"))  # PE clock-gate warmup matmuls
USE_TTR = os.environ.get("K_TTR", "0") == "1"
POOL_STT = os.environ.get("K_POOLSTT", "0") == "1"
SPLIT_DMA = os.environ.get("K_SPLITDMA", "1") == "1"

_NC_CACHE = {}


def _segs():
    """Per projection d-tile: (psum_row, head, head_row, nrows) segments
    mapping packed d rows (128*dt + p) onto per-head (h, q<96) layout."""
    segs = []
    for dt in range(DT):
        cur, d0, d1 = [], 128 * dt, 128 * (dt + 1)
        d = d0
        while d < d1:
            h, q = d // DH, d % DH
            n = min(d1 - d, DH - q)
            cur.append((d - d0, h, q, n))
            d += n
        segs.append(cur)
    return segs


def _build_nc():
    import concourse.bacc as bacc
    import concourse.tile as tile
    from concourse import mybir

    f32 = mybir.dt.float32
    bf16 = mybir.dt.bfloat16
    AF = mybir.ActivationFunctionType
    OP = mybir.AluOpType
    AX = mybir.AxisListType

    nc = bacc.Bacc()
    xt = nc.declare_dram_parameter("xt", [BPC, KT, 128, S], bf16, isOutput=False)
    p_l = nc.declare_dram_parameter("p_l", [128, KT, NH * DH], bf16, isOutput=False)
    w1 = nc.declare_dram_parameter("w1", [97, NH, DHID], bf16, isOutput=False)
    w2 = nc.declare_dram_parameter("w2", [128, NH, KC, DH], bf16, isOutput=False)
    pb = nc.declare_dram_parameter("pb", [128, DT], f32, isOutput=False)
    w2b = nc.declare_dram_parameter("w2b", [DH, NH], f32, isOutput=False)
    ones = nc.declare_dram_parameter("ones", [1, NH, S], bf16, isOutput=False)
    out = nc.declare_dram_parameter("out", [DH, BPC * NH], f32, isOutput=True)

    segs = _segs()

    with tile.TileContext(nc) as tc:
        with ExitStack() as ctx:
            singles = ctx.enter_context(tc.tile_pool(name="singles", bufs=1))
            xt_pool = ctx.enter_context(tc.tile_pool(name="xtp", bufs=2))
            flat_pool = ctx.enter_context(tc.tile_pool(name="flat", bufs=7))
            a1sb_pool = ctx.enter_context(tc.tile_pool(name="a1sb", bufs=4))
            e_pool = ctx.enter_context(tc.tile_pool(name="ep", bufs=3))
            stt_pool = ctx.enter_context(tc.tile_pool(name="sttp", bufs=2))
            small_pool = ctx.enter_context(tc.tile_pool(name="small", bufs=4))
            pp_pool = ctx.enter_context(tc.tile_pool(name="pp", bufs=3, space="PSUM"))
            a1p_pool = ctx.enter_context(tc.tile_pool(name="a1p", bufs=3, space="PSUM"))
            a2p_pool = ctx.enter_context(tc.tile_pool(name="a2p", bufs=2, space="PSUM"))

            # PE warmup: dummy matmuls on a memset tile while DMAs stream in.
            warm_sb = singles.tile([128, 128], bf16)
            warm_sb2 = singles.tile([128, 128], bf16)
            nc.vector.memset(warm_sb, 0.0)
            nc.vector.memset(warm_sb2, 0.0)
            for i in range(N_WARM):
                wp = pp_pool.tile([128, SCW], f32, tag="pp")
                nc.tensor.matmul(
                    wp[:, 0:128], warm_sb, warm_sb2, start=True, stop=True
                )

            # Projection inputs first (needed immediately), per-kt granular.
            p_sb = singles.tile([128, KT, NH * DH], bf16)
            xt_t0 = xt_pool.tile([128, KT, S], bf16, tag="xt0")
            if SPLIT_DMA:
                for kt in range(KT):
                    nc.sync.dma_start(out=p_sb[:, kt, :], in_=p_l[:, kt, :])
                    nc.sync.dma_start(
                        out=xt_t0[:, kt, 0:1024], in_=xt[0, kt, :, 0:1024]
                    )
                for kt in range(KT):
                    nc.sync.dma_start(
                        out=xt_t0[:, kt, 1024:S], in_=xt[0, kt, :, 1024:S]
                    )
            else:
                nc.sync.dma_start(out=p_sb, in_=p_l[:])
                nc.sync.dma_start(out=xt_t0, in_=xt[0].rearrange("kt p s -> p kt s"))
            pb_sb = singles.tile([128, DT], f32)
            nc.sync.dma_start(out=pb_sb, in_=pb[:])

            # MLP-phase constants (needed ~30us in).
            w1_sb = singles.tile([97, NH, DHID], bf16)
            nc.sync.dma_start(out=w1_sb, in_=w1[:])
            w2_sb = singles.tile([128, NH, KC, DH], bf16)
            nc.sync.dma_start(out=w2_sb, in_=w2[:])
            w2b_sb = singles.tile([DH, NH], f32)
            nc.sync.dma_start(out=w2b_sb, in_=w2b[:])
            v_sb = singles.tile([DH, BPC * NH], f32)
            zr_sb = singles.tile([DH, BPC * NH], f32)
            vn_sb = singles.tile([DH, BPC * NH], f32)
            lastacc = singles.tile([DH, 2], f32)

            # Hi^T in per-head layout; row 96 is a constant ones row that
            # realizes the W1 bias as a 97th contraction row. Two manual
            # buffers so batch b+1's projection overlaps batch b's MLP.
            hh = []
            for i in range(2):
                t = singles.tile([97, NH, S], bf16, tag=f"hh{i}")
                nc.sync.dma_start(out=t[96:97, :, :], in_=ones[:])
                hh.append(t)

            def proj_quarter(b, xt_t, dt, sc, flat_t):
                ps = pp_pool.tile([128, SCW], f32, tag="pp")
                for kt in range(KT):
                    nc.tensor.matmul(
                        ps,
                        p_sb[:, kt, 128 * dt:128 * (dt + 1)],
                        xt_t[:, kt, SCW * sc:SCW * (sc + 1)],
                        start=(kt == 0),
                        stop=(kt == KT - 1),
                    )
                if b == 0 and sc < 2:
                    nc.scalar.activation(
                        out=flat_t[:, SCW * sc:SCW * (sc + 1)],
                        in_=ps,
                        func=AF.Identity,
                        bias=pb_sb[:, dt:dt + 1],
                    )
                else:
                    nc.vector.tensor_scalar_add(
                        out=flat_t[:, SCW * sc:SCW * (sc + 1)],
                        in0=ps,
                        scalar1=pb_sb[:, dt:dt + 1],
                    )

            def emit_remap_dtile(b, dt, flat_t, lo=0, hi=S):
                hcur = hh[b % 2]
                for (r0, h, q0, n) in segs[dt]:
                    nc.sync.dma_start(
                        out=hcur[q0:q0 + n, h, lo:hi],
                        in_=flat_t[r0:r0 + n, lo:hi],
                    )

            def mlp_quarter(b, h, sc, e_t, zp, flush=False):
                hcur = hh[b % 2]
                a1sb = a1sb_pool.tile([128, KC, SCW], bf16)
                for c in range(KC):
                    a1p = a1p_pool.tile([128, SCW], f32, tag="a1p")
                    nc.tensor.matmul(
                        a1p,
                        w1_sb[:, h, 128 * c:128 * (c + 1)],
                        hcur[:, h, SCW * sc:SCW * (sc + 1)],
                        start=True,
                        stop=True,
                    )
                    if flush:
                        on_act = c == 0
                    else:
                        on_act = c == 0 or (c == 1 and sc != 3)
                    if on_act:
                        nc.scalar.activation(
                            out=a1sb[:, c, :], in_=a1p, func=AF.Relu
                        )
                    else:
                        nc.vector.tensor_scalar_max(
                            out=a1sb[:, c, :], in0=a1p, scalar1=0.0
                        )
                a2p = a2p_pool.tile([DH, SCW], f32, tag="a2p")
                for kc in range(KC):
                    nc.tensor.matmul(
                        a2p,
                        w2_sb[:, h, kc, :],
                        a1sb[:, kc, :],
                        start=(kc == 0),
                        stop=(kc == KC - 1),
                    )
                nc.scalar.activation(
                    out=e_t[:, SCW * sc:SCW * (sc + 1)],
                    in_=a2p,
                    func=AF.Exp,
                    bias=w2b_sb[:, h:h + 1],
                    accum_out=zp[:, sc:sc + 1],
                )
                # The very last head pools its first half early so the
                # closing pooling op is half-length (shorter kernel tail).
                if (b, h, sc) == (BPC - 1, NH - 1, 1):
                    stt_t = stt_pool.tile([DH, 1024], bf16, tag="stth", name="stth")
                    nc.vector.scalar_tensor_tensor(
                        out=stt_t,
                        in0=e_t[:, 0:1024],
                        scalar=1.0,
                        in1=hcur[0:DH, h, 0:1024],
                        op0=OP.mult,
                        op1=OP.mult,
                        accum_out=lastacc[:, 0:1],
                    )

            def mlp_finish(b, h, e_t):
                hcur = hh[b % 2]
                col = b * NH + h
                z1 = small_pool.tile([DH, 1], f32, tag="z1")
                nc.vector.tensor_reduce(
                    out=z1, in_=small_state[(b, h)][1], axis=AX.X, op=OP.add
                )
                nc.vector.reciprocal(zr_sb[:, col:col + 1], z1)
                if (b, h) == (BPC - 1, NH - 1):
                    stt_t = stt_pool.tile([DH, 1024], bf16, tag="stth", name="stth")
                    nc.vector.scalar_tensor_tensor(
                        out=stt_t,
                        in0=e_t[:, 1024:S],
                        scalar=1.0,
                        in1=hcur[0:DH, h, 1024:S],
                        op0=OP.mult,
                        op1=OP.mult,
                        accum_out=lastacc[:, 1:2],
                    )
                    nc.vector.tensor_reduce(
                        out=v_sb[:, col:col + 1], in_=lastacc, axis=AX.X,
                        op=OP.add,
                    )
                else:
                    stt_t = stt_pool.tile([DH, S], bf16)
                    eng = nc.gpsimd if (POOL_STT and h % 2 == 1) else nc.vector
                    eng.scalar_tensor_tensor(
                        out=stt_t,
                        in0=e_t,
                        scalar=1.0,
                        in1=hcur[0:DH, h, :],
                        op0=OP.mult,
                        op1=OP.mult,
                        accum_out=v_sb[:, col:col + 1],
                    )
                if h == NH - 1:
                    c0, c1 = b * NH, (b + 1) * NH
                    nc.vector.tensor_mul(
                        vn_sb[:, c0:c1], v_sb[:, c0:c1], zr_sb[:, c0:c1]
                    )
                    nc.sync.dma_start(out=out[:, c0:c1], in_=vn_sb[:, c0:c1])

            # --- ratio-paced scheduler over quarter-granularity units ---
            # P units: (b, dt, sc) projection quarters; M units: (b, h, sc)
            # MLP quarters. Pace M:P at 32:24 per batch so ACT/DVE load
            # stays near its average; M gated on its head's remaps.
            dts_of = {}
            for h in range(NH):
                dts_of[h] = sorted({(DH * h) // 128, (DH * h + DH - 1) // 128})
            # Batch 0 runs sc0/sc1 for all dtiles first (the DMA loads
            # first-halves of X^T first, so real work starts ~5us sooner),
            # then sc2/sc3; its Hi rows are remapped per half so MLP units
            # unlock during the first block. Later batches are dt-major
            # with full remaps (their X^T is fully prefetched).
            P_units = [(0, dt, sc) for dt in range(DT) for sc in (0, 1)]
            P_units += [(0, dt, sc) for dt in range(DT) for sc in (2, 3)]
            P_units += [(b, dt, sc) for b in range(1, BPC) for dt in range(DT)
                        for sc in range(SC)]
            M_units = [(b, h, sc) for b in range(BPC) for h in range(NH)
                       for sc in range(SC)]
            flat_state = {}
            small_state = {}
            remapped = set()
            xt_tiles = {0: xt_t0}
            p_i = m_i = 0

            def emit_P():
                nonlocal p_i
                b, dt, sc = P_units[p_i]
                if dt == 2 and sc == 0 and b + 1 < BPC:
                    nxt = xt_pool.tile([128, KT, S], bf16, tag="xt0")
                    for kt in range(KT):
                        nc.sync.dma_start(out=nxt[:, kt, :], in_=xt[b + 1, kt])
                    xt_tiles[b + 1] = nxt
                if (b, dt) not in flat_state:
                    flat_state[(b, dt)] = flat_pool.tile([128, S], bf16, tag="flat", name="flat_t")
                proj_quarter(b, xt_tiles[b], dt, sc, flat_state[(b, dt)])
                if b == 0 and sc == 1:
                    emit_remap_dtile(b, dt, flat_state[(b, dt)], 0, 1024)
                    remapped.add((0, dt, 0))
                elif sc == SC - 1:
                    lo = 1024 if b == 0 else 0
                    emit_remap_dtile(b, dt, flat_state.pop((b, dt)), lo, S)
                    remapped.add((b, dt))
                p_i += 1

            def emit_M():
                nonlocal m_i
                b, h, sc = M_units[m_i]
                if sc == 0:
                    small_state[(b, h)] = (
                        e_pool.tile([DH, S], bf16, tag="e_t", name="e_t"),
                        small_pool.tile([DH, SC], f32, tag="zp", name="zp"),
                    )
                e_t, zp = small_state[(b, h)]
                mlp_quarter(b, h, sc, e_t, zp, flush=(p_i >= len(P_units)))
                if sc == SC - 1:
                    mlp_finish(b, h, e_t)
                    del small_state[(b, h)]
                m_i += 1

            def m_ready():
                if m_i >= len(M_units):
                    return False
                b, h, sc = M_units[m_i]
                if b == 0 and sc < 2:
                    return all(
                        (0, dt) in remapped or (0, dt, 0) in remapped
                        for dt in dts_of[h]
                    )
                return all((b, dt) in remapped for dt in dts_of[h])

            LEAD = 8  # projection quarters of head start
            while p_i < len(P_units) or m_i < len(M_units):
                lead = LEAD if p_i < len(P_units) - 16 else 0
                want_m = m_i * 3 <= (p_i - lead) * 4
                if p_i < len(P_units) and not (want_m and m_ready()):
                    emit_P()
                elif m_ready():
                    emit_M()
                elif p_i < len(P_units):
                    emit_P()
                else:
                    # only unready M left: emit in order anyway (deps safe)
                    emit_M()

    nc.compile()
    return nc


def get_nc():
    if "nc" not in _NC_CACHE:
        _NC_CACHE["nc"] = _build_nc()
    return _NC_CACHE["nc"]


def make_in_maps(token_embeddings, P_w, P_b, W1_w, W1_b, W2_w, W2_b):
    import ml_dtypes

    bf16 = ml_dtypes.bfloat16
    X = np.asarray(token_embeddings, dtype=np.float32)
    # X^T per batch: (B, T, S) -> tiles [b, kt, p, s]
    XT = np.ascontiguousarray(X.transpose(0, 2, 1)).astype(bf16)
    XT = XT.reshape(B, KT, 128, S)

    P_cat = np.transpose(np.asarray(P_w, np.float32), (1, 0, 2)).reshape(T, NH * DH)
    p_l = np.ascontiguousarray(
        P_cat.reshape(KT, 128, NH * DH).transpose(1, 0, 2)
    ).astype(bf16)

    w1 = np.zeros((97, NH, DHID), dtype=bf16)
    w1[:96] = np.asarray(W1_w, np.float32).transpose(1, 0, 2).astype(bf16)
    w1[96] = np.asarray(W1_b, np.float32).astype(bf16)

    w2 = np.ascontiguousarray(
        np.asarray(W2_w, np.float32).reshape(NH, KC, 128, DH).transpose(2, 0, 1, 3)
    ).astype(bf16)

    pb = np.ascontiguousarray(
        np.asarray(P_b, np.float32).reshape(NH * DH).reshape(KT, 128).T
    ).astype(np.float32)
    w2b = np.ascontiguousarray(np.asarray(W2_b, np.float32).T)
    ones = np.ones((1, NH, S), dtype=bf16)

    in_maps = []
    for c in range(NCORES):
        in_maps.append({
            "xt": np.ascontiguousarray(XT[c * BPC:(c + 1) * BPC]),
            "p_l": p_l,
            "w1": w1,
            "w2": w2,
            "pb": pb,
            "w2b": w2b,
            "ones": ones,
        })
    return in_maps


def _reference_host(token_embeddings, attention_mask, P_w, P_b, W1_w, W1_b, W2_w, W2_b):
    """Exact numpy fallback (only used if the mask is not all-ones)."""
    X = np.asarray(token_embeddings, np.float64)
    Hi = np.einsum("bst,htd->bhsd", X, np.asarray(P_w, np.float64))
    Hi += np.asarray(P_b, np.float64)[None, :, None, :]
    A = np.einsum("bhsd,hde->bhse", Hi, np.asarray(W1_w, np.float64))
    A += np.asarray(W1_b, np.float64)[None, :, None, :]
    A = np.maximum(A, 0.0)
    A = np.einsum("bhse,hed->bhsd", A, np.asarray(W2_w, np.float64))
    A += np.asarray(W2_b, np.float64)[None, :, None, :]
    with np.errstate(divide="ignore"):
        logm = np.log(np.asarray(attention_mask, np.float64))[:, None, :, None]
    A = A + logm
    A = A - A.max(axis=2, keepdims=True)
    E = np.exp(A)
    A = E / E.sum(axis=2, keepdims=True)
    v = (Hi * A).sum(axis=2)
    return v.reshape(v.shape[0], NH * DH).astype(np.float32)


def kernel(**inputs):
    mask = np.asarray(inputs["attention_mask"], np.float32)
    if not np.all(mask == 1.0):
        return _reference_host(**inputs)

    from concourse.bass_utils import run_bass_kernel_spmd

    nc = get_nc()
    in_maps = make_in_maps(
        inputs["token_embeddings"], inputs["P_w"], inputs["P_b"],
        inputs["W1_w"], inputs["W1_b"], inputs["W2_w"], inputs["W2_b"],
    )
    res = run_bass_kernel_spmd(nc, in_maps, core_ids=list(range(NCORES)))
    outs = [
        np.asarray(r["out"], np.float32).T.reshape(BPC, NH * DH)
        for r in res.results
    ]
    return np.concatenate(outs, axis=0)



# revision 37
# speedup vs baseline: 1.0023x; 1.0023x over previous
"""MultiHeadGeneralizedPooling Trainium2 kernel.

Data-parallel over batch: 32 batches -> 8 cores x 4 batches.
Per core, everything is computed in "feature-major" layout (feature dim on
SBUF partitions, sequence on the free axis):

  Hi^T (d, s)  = P_cat^T @ X^T          TensorE bf16, PSUM; +P_b on copy-out
  A1^T (dh, s) = relu(W1aug^T @ Hi^T)   K=97 (97th row = ones -> W1 bias)
  A2^T (d, s)  = W2^T @ A1^T            accumulated over 3 k-tiles
  E            = exp(A2^T + W2_b)       ScalarE; accum_out -> Z per partition
  u[d]         = sum_s E * Hi           fused DVE tensor_tensor_reduce accum
  v            = u / Z                  one tiny (96, 32) multiply at the end

Host side pre-transposes/casts X to X^T bf16 and packs the (tiny) weights
into lhsT layouts, so the device does no transposes of the big tensor.
"""

import numpy as np
from contextlib import ExitStack

B, S, T = 32, 2048, 768
NH, DH, DHID = 8, 96, 384
NCORES = 8
BPC = B // NCORES  # batches per core
KT = T // 128      # 6 contraction tiles
DT = (NH * DH) // 128  # 6 d-tiles of the packed head dim
SC = 4             # s-chunks per batch
SCW = S // SC      # 512
KC = DHID // 128   # 3
import os
N_WARM = int(os.environ.get("K_NWARM", "45"))  # PE clock-gate warmup matmuls
USE_TTR = os.environ.get("K_TTR", "0") == "1"
POOL_STT = os.environ.get("K_POOLSTT", "0") == "1"
SPLIT_DMA = os.environ.get("K_SPLITDMA", "1") == "1"

_NC_CACHE = {}


def _segs():
    """Per projection d-tile: (psum_row, head, head_row, nrows) segments
    mapping packed d rows (128*dt + p) onto per-head (h, q<96) layout."""
    segs = []
    for dt in range(DT):
        cur, d0, d1 = [], 128 * dt, 128 * (dt + 1)
        d = d0
        while d < d1:
            h, q = d // DH, d % DH
            n = min(d1 - d, DH - q)
            cur.append((d - d0, h, q, n))
            d += n
        segs.append(cur)
    return segs


def _build_nc():
    import concourse.bacc as bacc
    import concourse.tile as tile
    from concourse import mybir

    f32 = mybir.dt.float32
    bf16 = mybir.dt.bfloat16
    AF = mybir.ActivationFunctionType
    OP = mybir.AluOpType
    AX = mybir.AxisListType

    nc = bacc.Bacc()
    xt = nc.declare_dram_parameter("xt", [BPC, KT, 128, S], bf16, isOutput=False)
    p_l = nc.declare_dram_parameter("p_l", [128, KT, NH * DH], bf16, isOutput=False)
    w1 = nc.declare_dram_parameter("w1", [97, NH, DHID], bf16, isOutput=False)
    w2 = nc.declare_dram_parameter("w2", [128, NH, KC, DH], bf16, isOutput=False)
    pb = nc.declare_dram_parameter("pb", [128, DT], f32, isOutput=False)
    w2b = nc.declare_dram_parameter("w2b", [DH, NH], f32, isOutput=False)
    ones = nc.declare_dram_parameter("ones", [1, NH, S], bf16, isOutput=False)
    out = nc.declare_dram_parameter("out", [DH, BPC * NH], f32, isOutput=True)

    segs = _segs()

    with tile.TileContext(nc) as tc:
        with ExitStack() as ctx:
            singles = ctx.enter_context(tc.tile_pool(name="singles", bufs=1))
            xt_pool = ctx.enter_context(tc.tile_pool(name="xtp", bufs=2))
            flat_pool = ctx.enter_context(tc.tile_pool(name="flat", bufs=7))
            a1sb_pool = ctx.enter_context(tc.tile_pool(name="a1sb", bufs=4))
            e_pool = ctx.enter_context(tc.tile_pool(name="ep", bufs=3))
            stt_pool = ctx.enter_context(tc.tile_pool(name="sttp", bufs=2))
            small_pool = ctx.enter_context(tc.tile_pool(name="small", bufs=4))
            pp_pool = ctx.enter_context(tc.tile_pool(name="pp", bufs=3, space="PSUM"))
            a1p_pool = ctx.enter_context(tc.tile_pool(name="a1p", bufs=3, space="PSUM"))
            a2p_pool = ctx.enter_context(tc.tile_pool(name="a2p", bufs=2, space="PSUM"))

            # PE warmup: dummy matmuls on a memset tile while DMAs stream in.
            warm_sb = singles.tile([128, 128], bf16)
            warm_sb2 = singles.tile([128, 128], bf16)
            nc.vector.memset(warm_sb, 0.0)
            nc.vector.memset(warm_sb2, 0.0)
            for i in range(N_WARM):
                wp = pp_pool.tile([128, SCW], f32, tag="pp")
                nc.tensor.matmul(
                    wp[:, 0:128], warm_sb, warm_sb2, start=True, stop=True
                )

            # Projection inputs first (needed immediately), per-kt granular.
            p_sb = singles.tile([128, KT, NH * DH], bf16)
            xt_t0 = xt_pool.tile([128, KT, S], bf16, tag="xt0")
            if SPLIT_DMA:
                for kt in range(KT):
                    nc.sync.dma_start(out=p_sb[:, kt, :], in_=p_l[:, kt, :])
                    nc.sync.dma_start(
                        out=xt_t0[:, kt, 0:1024], in_=xt[0, kt, :, 0:1024]
                    )
                for kt in range(KT):
                    nc.sync.dma_start(
                        out=xt_t0[:, kt, 1024:S], in_=xt[0, kt, :, 1024:S]
                    )
            else:
                nc.sync.dma_start(out=p_sb, in_=p_l[:])
                nc.sync.dma_start(out=xt_t0, in_=xt[0].rearrange("kt p s -> p kt s"))
            pb_sb = singles.tile([128, DT], f32)
            nc.sync.dma_start(out=pb_sb, in_=pb[:])

            # MLP-phase constants (needed ~30us in).
            w1_sb = singles.tile([97, NH, DHID], bf16)
            nc.sync.dma_start(out=w1_sb, in_=w1[:])
            w2_sb = singles.tile([128, NH, KC, DH], bf16)
            nc.sync.dma_start(out=w2_sb, in_=w2[:])
            w2b_sb = singles.tile([DH, NH], f32)
            nc.sync.dma_start(out=w2b_sb, in_=w2b[:])
            v_sb = singles.tile([DH, BPC * NH], f32)
            zr_sb = singles.tile([DH, BPC * NH], f32)
            vn_sb = singles.tile([DH, BPC * NH], f32)
            lastacc = singles.tile([DH, 2], f32)

            # Hi^T in per-head layout; row 96 is a constant ones row that
            # realizes the W1 bias as a 97th contraction row. Two manual
            # buffers so batch b+1's projection overlaps batch b's MLP.
            hh = []
            for i in range(2):
                t = singles.tile([97, NH, S], bf16, tag=f"hh{i}")
                nc.sync.dma_start(out=t[96:97, :, :], in_=ones[:])
                hh.append(t)

            def proj_quarter(b, xt_t, dt, sc, flat_t):
                ps = pp_pool.tile([128, SCW], f32, tag="pp")
                for kt in range(KT):
                    nc.tensor.matmul(
                        ps,
                        p_sb[:, kt, 128 * dt:128 * (dt + 1)],
                        xt_t[:, kt, SCW * sc:SCW * (sc + 1)],
                        start=(kt == 0),
                        stop=(kt == KT - 1),
                    )
                # Always DVE: it is table-free and ready ~6us in, while
                # ACT's first op stalls ~25us on the static-DMA'd ACT
                # tables. (ACT still carries most relus + all exps later.)
                nc.vector.tensor_scalar_add(
                    out=flat_t[:, SCW * sc:SCW * (sc + 1)],
                    in0=ps,
                    scalar1=pb_sb[:, dt:dt + 1],
                )

            def emit_remap_dtile(b, dt, flat_t, lo=0, hi=S):
                hcur = hh[b % 2]
                for (r0, h, q0, n) in segs[dt]:
                    nc.sync.dma_start(
                        out=hcur[q0:q0 + n, h, lo:hi],
                        in_=flat_t[r0:r0 + n, lo:hi],
                    )

            def mlp_quarter(b, h, sc, e_t, zp, flush=False):
                hcur = hh[b % 2]
                a1sb = a1sb_pool.tile([128, KC, SCW], bf16)
                for c in range(KC):
                    a1p = a1p_pool.tile([128, SCW], f32, tag="a1p")
                    nc.tensor.matmul(
                        a1p,
                        w1_sb[:, h, 128 * c:128 * (c + 1)],
                        hcur[:, h, SCW * sc:SCW * (sc + 1)],
                        start=True,
                        stop=True,
                    )
                    if flush:
                        on_act = c == 0
                    else:
                        on_act = c == 0 or (c == 1 and sc != 3)
                    if on_act:
                        nc.scalar.activation(
                            out=a1sb[:, c, :], in_=a1p, func=AF.Relu
                        )
                    else:
                        nc.vector.tensor_scalar_max(
                            out=a1sb[:, c, :], in0=a1p, scalar1=0.0
                        )
                a2p = a2p_pool.tile([DH, SCW], f32, tag="a2p")
                for kc in range(KC):
                    nc.tensor.matmul(
                        a2p,
                        w2_sb[:, h, kc, :],
                        a1sb[:, kc, :],
                        start=(kc == 0),
                        stop=(kc == KC - 1),
                    )
                nc.scalar.activation(
                    out=e_t[:, SCW * sc:SCW * (sc + 1)],
                    in_=a2p,
                    func=AF.Exp,
                    bias=w2b_sb[:, h:h + 1],
                    accum_out=zp[:, sc:sc + 1],
                )
                # The very last head pools its first half early so the
                # closing pooling op is half-length (shorter kernel tail).
                if (b, h, sc) == (BPC - 1, NH - 1, 1):
                    stt_t = stt_pool.tile([DH, 1024], bf16, tag="stth", name="stth")
                    nc.vector.scalar_tensor_tensor(
                        out=stt_t,
                        in0=e_t[:, 0:1024],
                        scalar=1.0,
                        in1=hcur[0:DH, h, 0:1024],
                        op0=OP.mult,
                        op1=OP.mult,
                        accum_out=lastacc[:, 0:1],
                    )

            def mlp_finish(b, h, e_t):
                hcur = hh[b % 2]
                col = b * NH + h
                z1 = small_pool.tile([DH, 1], f32, tag="z1")
                nc.vector.tensor_reduce(
                    out=z1, in_=small_state[(b, h)][1], axis=AX.X, op=OP.add
                )
                nc.vector.reciprocal(zr_sb[:, col:col + 1], z1)
                if (b, h) == (BPC - 1, NH - 1):
                    stt_t = stt_pool.tile([DH, 1024], bf16, tag="stth", name="stth")
                    nc.vector.scalar_tensor_tensor(
                        out=stt_t,
                        in0=e_t[:, 1024:S],
                        scalar=1.0,
                        in1=hcur[0:DH, h, 1024:S],
                        op0=OP.mult,
                        op1=OP.mult,
                        accum_out=lastacc[:, 1:2],
                    )
                    nc.vector.tensor_reduce(
                        out=v_sb[:, col:col + 1], in_=lastacc, axis=AX.X,
                        op=OP.add,
                    )
                else:
                    stt_t = stt_pool.tile([DH, S], bf16)
                    eng = nc.gpsimd if (POOL_STT and h % 2 == 1) else nc.vector
                    eng.scalar_tensor_tensor(
                        out=stt_t,
                        in0=e_t,
                        scalar=1.0,
                        in1=hcur[0:DH, h, :],
                        op0=OP.mult,
                        op1=OP.mult,
                        accum_out=v_sb[:, col:col + 1],
                    )
                if h == NH - 1:
                    c0, c1 = b * NH, (b + 1) * NH
                    nc.vector.tensor_mul(
                        vn_sb[:, c0:c1], v_sb[:, c0:c1], zr_sb[:, c0:c1]
                    )
                    nc.sync.dma_start(out=out[:, c0:c1], in_=vn_sb[:, c0:c1])

            # --- ratio-paced scheduler over quarter-granularity units ---
            # P units: (b, dt, sc) projection quarters; M units: (b, h, sc)
            # MLP quarters. Pace M:P at 32:24 per batch so ACT/DVE load
            # stays near its average; M gated on its head's remaps.
            dts_of = {}
            for h in range(NH):
                dts_of[h] = sorted({(DH * h) // 128, (DH * h + DH - 1) // 128})
            # Batch 0 runs sc0/sc1 for all dtiles first (the DMA loads
            # first-halves of X^T first, so real work starts ~5us sooner),
            # then sc2/sc3; its Hi rows are remapped per half so MLP units
            # unlock during the first block. Later batches are dt-major
            # with full remaps (their X^T is fully prefetched).
            P_units = [(0, dt, sc) for dt in range(DT) for sc in (0, 1)]
            P_units += [(0, dt, sc) for dt in range(DT) for sc in (2, 3)]
            P_units += [(b, dt, sc) for b in range(1, BPC) for dt in range(DT)
                        for sc in range(SC)]
            M_units = [(b, h, sc) for b in range(BPC) for h in range(NH)
                       for sc in range(SC)]
            flat_state = {}
            small_state = {}
            remapped = set()
            xt_tiles = {0: xt_t0}
            p_i = m_i = 0

            def emit_P():
                nonlocal p_i
                b, dt, sc = P_units[p_i]
                if dt == 2 and sc == 0 and b + 1 < BPC:
                    nxt = xt_pool.tile([128, KT, S], bf16, tag="xt0")
                    for kt in range(KT):
                        nc.sync.dma_start(out=nxt[:, kt, :], in_=xt[b + 1, kt])
                    xt_tiles[b + 1] = nxt
                if (b, dt) not in flat_state:
                    flat_state[(b, dt)] = flat_pool.tile([128, S], bf16, tag="flat", name="flat_t")
                proj_quarter(b, xt_tiles[b], dt, sc, flat_state[(b, dt)])
                if b == 0 and sc == 1:
                    emit_remap_dtile(b, dt, flat_state[(b, dt)], 0, 1024)
                    remapped.add((0, dt, 0))
                elif sc == SC - 1:
                    lo = 1024 if b == 0 else 0
                    emit_remap_dtile(b, dt, flat_state.pop((b, dt)), lo, S)
                    remapped.add((b, dt))
                p_i += 1

            def emit_M():
                nonlocal m_i
                b, h, sc = M_units[m_i]
                if sc == 0:
                    small_state[(b, h)] = (
                        e_pool.tile([DH, S], bf16, tag="e_t", name="e_t"),
                        small_pool.tile([DH, SC], f32, tag="zp", name="zp"),
                    )
                e_t, zp = small_state[(b, h)]
                mlp_quarter(b, h, sc, e_t, zp, flush=(p_i >= len(P_units)))
                if sc == SC - 1:
                    mlp_finish(b, h, e_t)
                    del small_state[(b, h)]
                m_i += 1

            def m_ready():
                if m_i >= len(M_units):
                    return False
                b, h, sc = M_units[m_i]
                if b == 0 and sc < 2:
                    return all(
                        (0, dt) in remapped or (0, dt, 0) in remapped
                        for dt in dts_of[h]
                    )
                return all((b, dt) in remapped for dt in dts_of[h])

            LEAD = 8  # projection quarters of head start
            while p_i < len(P_units) or m_i < len(M_units):
                lead = LEAD if p_i < len(P_units) - 16 else 0
                want_m = m_i * 3 <= (p_i - lead) * 4
                if p_i < len(P_units) and not (want_m and m_ready()):
                    emit_P()
                elif m_ready():
                    emit_M()
                elif p_i < len(P_units):
                    emit_P()
                else:
                    # only unready M left: emit in order anyway (deps safe)
                    emit_M()

    nc.compile()
    return nc


def get_nc():
    if "nc" not in _NC_CACHE:
        _NC_CACHE["nc"] = _build_nc()
    return _NC_CACHE["nc"]


def make_in_maps(token_embeddings, P_w, P_b, W1_w, W1_b, W2_w, W2_b):
    import ml_dtypes

    bf16 = ml_dtypes.bfloat16
    X = np.asarray(token_embeddings, dtype=np.float32)
    # X^T per batch: (B, T, S) -> tiles [b, kt, p, s]
    XT = np.ascontiguousarray(X.transpose(0, 2, 1)).astype(bf16)
    XT = XT.reshape(B, KT, 128, S)

    P_cat = np.transpose(np.asarray(P_w, np.float32), (1, 0, 2)).reshape(T, NH * DH)
    p_l = np.ascontiguousarray(
        P_cat.reshape(KT, 128, NH * DH).transpose(1, 0, 2)
    ).astype(bf16)

    w1 = np.zeros((97, NH, DHID), dtype=bf16)
    w1[:96] = np.asarray(W1_w, np.float32).transpose(1, 0, 2).astype(bf16)
    w1[96] = np.asarray(W1_b, np.float32).astype(bf16)

    w2 = np.ascontiguousarray(
        np.asarray(W2_w, np.float32).reshape(NH, KC, 128, DH).transpose(2, 0, 1, 3)
    ).astype(bf16)

    pb = np.ascontiguousarray(
        np.asarray(P_b, np.float32).reshape(NH * DH).reshape(KT, 128).T
    ).astype(np.float32)
    w2b = np.ascontiguousarray(np.asarray(W2_b, np.float32).T)
    ones = np.ones((1, NH, S), dtype=bf16)

    in_maps = []
    for c in range(NCORES):
        in_maps.append({
            "xt": np.ascontiguousarray(XT[c * BPC:(c + 1) * BPC]),
            "p_l": p_l,
            "w1": w1,
            "w2": w2,
            "pb": pb,
            "w2b": w2b,
            "ones": ones,
        })
    return in_maps


def _reference_host(token_embeddings, attention_mask, P_w, P_b, W1_w, W1_b, W2_w, W2_b):
    """Exact numpy fallback (only used if the mask is not all-ones)."""
    X = np.asarray(token_embeddings, np.float64)
    Hi = np.einsum("bst,htd->bhsd", X, np.asarray(P_w, np.float64))
    Hi += np.asarray(P_b, np.float64)[None, :, None, :]
    A = np.einsum("bhsd,hde->bhse", Hi, np.asarray(W1_w, np.float64))
    A += np.asarray(W1_b, np.float64)[None, :, None, :]
    A = np.maximum(A, 0.0)
    A = np.einsum("bhse,hed->bhsd", A, np.asarray(W2_w, np.float64))
    A += np.asarray(W2_b, np.float64)[None, :, None, :]
    with np.errstate(divide="ignore"):
        logm = np.log(np.asarray(attention_mask, np.float64))[:, None, :, None]
    A = A + logm
    A = A - A.max(axis=2, keepdims=True)
    E = np.exp(A)
    A = E / E.sum(axis=2, keepdims=True)
    v = (Hi * A).sum(axis=2)
    return v.reshape(v.shape[0], NH * DH).astype(np.float32)


def kernel(**inputs):
    mask = np.asarray(inputs["attention_mask"], np.float32)
    if not np.all(mask == 1.0):
        return _reference_host(**inputs)

    from concourse.bass_utils import run_bass_kernel_spmd

    nc = get_nc()
    in_maps = make_in_maps(
        inputs["token_embeddings"], inputs["P_w"], inputs["P_b"],
        inputs["W1_w"], inputs["W1_b"], inputs["W2_w"], inputs["W2_b"],
    )
    res = run_bass_kernel_spmd(nc, in_maps, core_ids=list(range(NCORES)))
    outs = [
        np.asarray(r["out"], np.float32).T.reshape(BPC, NH * DH)
        for r in res.results
    ]
    return np.concatenate(outs, axis=0)

